# revision 1
# baseline (speedup 1.0000x reference)
"""CeATTForTCPFormer Trainium2 kernel (8 NeuronCores, data-parallel over B).

Contract: kernel(**inputs) takes FULL inputs as in reference.setup_inputs()
and returns the FULL [32, 243, 17, 256] fp32 output. Internally shards B
across 8 cores (4 per core); BN batch stats are combined with one small
AllReduce.
"""
import os
import sys

sys.path.insert(0, "/opt/trn_rl_repo")
sys.path.insert(0, "/opt/trn_rl_repo/concourse")

import numpy as np
import ml_dtypes

import concourse.bass as bass
import concourse.mybir as mybir
import concourse.tile as tile
from concourse.tile_rust import add_dep_helper
from concourse.bass_utils import run_bass_kernel_spmd

F32 = mybir.dt.float32
F32R = mybir.dt.float32r
BF16 = mybir.dt.bfloat16
AF = mybir.ActivationFunctionType
OP = mybir.AluOpType
AX = mybir.AxisListType

N_CORES = 8
B, T, J, C = 32, 243, 17, 256
BSH = B // N_CORES          # 4 batch elems per core
H, DH = 8, 32
LT = 81                     # temporal pooled length (243/3)
LS = 8                      # spatial pooled length (17//2)
NT_B = J                    # temporal seqs per batch elem
TOKT_B = J * LT             # 1377 temporal tokens per b
NS_B = T                    # spatial seqs per b (243)
TOKS_B = T * LS             # 1944 spatial tokens per b
CNT_T = float(B * J * LT)   # global BN count temporal = 44064
CNT_S = float(B * T * LS)   # spatial N = B*T, L = 8 -> 62208
SCALE = 1.0 / np.sqrt(DH)
EPS = 1e-5

# walrus in this container accepts at most 1 sync-wait command per
# instruction; Tile's tail drain carries one wait per logical processor.
MAX_WAITS = 1


def _split_excess_waits(nc):
    ctr = 0
    for f in nc.m.functions:
        for bb in f.blocks:
            new_insts, changed = [], False
            for inst in bb.instructions:
                si = inst.sync_info
                if si is not None and si.on_wait is not None and len(si.on_wait) > MAX_WAITS:
                    waits = list(si.on_wait)
                    upd = list(si.on_update or [])
                    rest, keep = waits[:-MAX_WAITS], waits[-MAX_WAITS:]
                    for w in rest:
                        nop = mybir.InstNoOp(name=f"waitsplit_{ctr}", ins=[], outs=[])
                        ctr += 1
                        nop.engine = inst.engine
                        nop.sync_info = mybir.SyncInfo(on_wait=[w], on_update=[])
                        new_insts.append(nop)
                    inst.sync_info = mybir.SyncInfo(on_wait=keep, on_update=upd)
                    changed = True
                new_insts.append(inst)
            if changed:
                bb.instructions = new_insts


def _interp_lin_coef(L, out_len):
    scale = L / out_len
    coords = (np.arange(out_len) + 0.5) * scale - 0.5
    coords = np.clip(coords, 0.0, L - 1)
    lo = np.floor(coords).astype(np.int32)
    hi = np.minimum(lo + 1, L - 1)
    w = (coords - lo).astype(np.float64)
    return lo, hi, w


def build(nc, dbg=()):
    """Emit the whole per-core program. Returns list of debug output names."""
    dbg = set(dbg)
    dbg_outs = []

    x_t = nc.dram_tensor("xs", [BSH, T, J, C], F32, kind="ExternalInput").ap()
    wqkv_t = nc.dram_tensor("wqkv_t", [C, 3 * C], BF16, kind="ExternalInput").ap()
    wqkv_s = nc.dram_tensor("wqkv_s", [C, 3 * C], BF16, kind="ExternalInput").ap()
    wproj_t = nc.dram_tensor("wproj_t", [C, C], BF16, kind="ExternalInput").ap()
    wproj_s = nc.dram_tensor("wproj_s", [C, C], BF16, kind="ExternalInput").ap()
    wpw_t = nc.dram_tensor("wpw_t", [C, C], BF16, kind="ExternalInput").ap()
    wpw_s = nc.dram_tensor("wpw_s", [C, C], BF16, kind="ExternalInput").ap()
    fw1_t = nc.dram_tensor("fw1", [2 * C, C], BF16, kind="ExternalInput").ap()
    fw2_t = nc.dram_tensor("fw2", [C, C], BF16, kind="ExternalInput").ap()
    pwf_t = nc.dram_tensor("pwf", [C, C], BF16, kind="ExternalInput").ap()
    idn_t = nc.dram_tensor("idn", [128, 128], BF16, kind="ExternalInput").ap()
    # vecs columns: 0-2 t_dw half0, 3-5 t_dw half1, 6-8 s_dw h0, 9-11 s_dw h1,
    # 12,13 t_bn_g h0/h1, 14,15 t_bn_b, 16,17 s_bn_g, 18,19 s_bn_b,
    # 20,21 f_b2, 22,23 p_b
    vecs_t = nc.dram_tensor("vecs", [128, 26], F32, kind="ExternalInput").ap()
    out_t = nc.dram_tensor("out", [BSH, T, J, C], F32, kind="ExternalOutput").ap()

    def dbg_out(name, shape, dtype=F32):
        ap = nc.dram_tensor("dbg_" + name, shape, dtype, kind="ExternalOutput").ap()
        dbg_outs.append("dbg_" + name)
        return ap

    tc = TCFix(nc)
    with tc:
        _build_body(nc, tc, locals(), dbg, dbg_out)
    _split_excess_waits(nc)
    return dbg_outs


class TCFix(tile.TileContext):
    pass  # tail-drain waits are split post-hoc by _split_excess_waits


PHASES = int(os.environ.get("KPHASES", "4"))


KSTEP = int(os.environ.get("KSTEP", "99"))
KATT = int(os.environ.get("KATT", "99"))


def _build_body(nc, tc, ctx, dbg, dbg_out):
    x_t = ctx["x_t"]; out_t = ctx["out_t"]; vecs_t = ctx["vecs_t"]
    wqkv_t = ctx["wqkv_t"]; wqkv_s = ctx["wqkv_s"]
    wproj_t = ctx["wproj_t"]; wproj_s = ctx["wproj_s"]
    wpw_t = ctx["wpw_t"]; wpw_s = ctx["wpw_s"]
    fw1_t = ctx["fw1_t"]; fw2_t = ctx["fw2_t"]; pwf_t = ctx["pwf_t"]
    idn_t = ctx["idn_t"]

    ex_cm = tc.tile_pool(name="ex", bufs=1)       # persistent: weights, stats
    ex = ex_cm.__enter__()
    dr_cm = tc.tile_pool(name="dr", bufs=1, space="DRAM")
    dr = dr_cm.__enter__()

    # ---- persistent weight tiles ----
    wqkvT = [ex.tile([128, 3 * C], BF16, name=f"wqkvT{k}") for k in range(2)]
    for k in range(2):
        nc.sync.dma_start(wqkvT[k][:], wqkv_t[128 * k:128 * (k + 1), :])
    wqkvS = [ex.tile([128, 3 * C], BF16, name=f"wqkvS{k}") for k in range(2)]
    for k in range(2):
        nc.sync.dma_start(wqkvS[k][:], wqkv_s[128 * k:128 * (k + 1), :])
    wprojT = [ex.tile([128, C], BF16, name=f"wprojT{k}") for k in range(2)]
    for k in range(2):
        nc.sync.dma_start(wprojT[k][:], wproj_t[128 * k:128 * (k + 1), :])
    wprojS = [ex.tile([128, C], BF16, name=f"wprojS{k}") for k in range(2)]
    for k in range(2):
        nc.sync.dma_start(wprojS[k][:], wproj_s[128 * k:128 * (k + 1), :])
    wpwT = [ex.tile([128, C], BF16, name=f"wpwT{k}") for k in range(2)]
    for k in range(2):
        nc.sync.dma_start(wpwT[k][:], wpw_t[128 * k:128 * (k + 1), :])
    wpwS = [ex.tile([128, C], BF16, name=f"wpwS{k}") for k in range(2)]
    for k in range(2):
        nc.sync.dma_start(wpwS[k][:], wpw_s[128 * k:128 * (k + 1), :])
    fw1T = [ex.tile([128, C], BF16, name=f"fw1T{k}") for k in range(4)]
    for k in range(4):
        nc.sync.dma_start(fw1T[k][:], fw1_t[128 * k:128 * (k + 1), :])
    fw2T = [ex.tile([128, C], BF16, name=f"fw2T{k}") for k in range(2)]
    for k in range(2):
        nc.sync.dma_start(fw2T[k][:], fw2_t[128 * k:128 * (k + 1), :])
    pwfT = [ex.tile([128, C], BF16, name=f"pwfT{k}") for k in range(2)]
    for k in range(2):
        nc.sync.dma_start(pwfT[k][:], pwf_t[128 * k:128 * (k + 1), :])
    idn = ex.tile([128, 128], BF16, name="idn")
    nc.sync.dma_start(idn[:], idn_t[:])
    vecs = ex.tile([128, 26], F32, name="vecs")
    nc.sync.dma_start(vecs[:], vecs_t[:])

    # spatial pooled input, bf16, built during temporal loop (uses X residency)
    xps = [ex.tile([128, BSH * TOKS_B], BF16, name=f"xps{k}") for k in range(2)]
    # BN partial accumulators: [t_sum h0,h1, t_sq h0,h1, s_sum h0,h1, s_sq h0,h1]
    accs = ex.tile([128, 8], F32, name="accs")
    nc.vector.memset(accs[:], 0.0)
    # temporal/spatial pre-BN activations parked in DRAM (bf16)
    yt_d = [dr.tile([128, BSH * TOKT_B], BF16, name=f"yt_d{k}") for k in range(2)]
    ys_d = [dr.tile([128, BSH * TOKS_B], BF16, name=f"ys_d{k}") for k in range(2)]

    # =================== PHASE A-t: temporal branch to pre-BN ===================
    with tc.tile_pool(name="pa", bufs=1) as pa, \
         tc.tile_pool(name="pa2", bufs=2) as pa2, \
         tc.tile_pool(name="pp", bufs=2, space="PSUM") as pp, \
         tc.tile_pool(name="pps", bufs=4, space="PSUM") as pps, \
         tc.tile_pool(name="ppo", bufs=2, space="PSUM") as ppo:
        for b in range(BSH):
            xpt = [pa.tile([128, TOKT_B], F32, tag=f"xpt{k}", name=f"xpt{b}_{k}") for k in range(2)]
            for k in range(2):
                xc = pa2.tile([128, T * J], F32, tag="xc", name=f"xc{b}_{k}")
                src = x_t[b].rearrange("t j c -> c (t j)")[128 * k:128 * (k + 1), :]
                nc.sync.dma_start(xc[:], src)
                # temporal pool: out[c, j*81+m] = sum_r x[c, (3m+r)*17+j]
                xv = xc[:].rearrange("p (t j) -> p j t", j=J)  # [128, j, 243]
                xv = xv.rearrange("p j (m r) -> p j m r", r=3)
                nc.vector.reduce_sum(xpt[k][:].rearrange("p (j m) -> p j m", j=J), xv, AX.X)
                # spatial pool: out[c, b*1944 + t*8 + l] = sum_r x[c, t*17 + 2l+r]
                xv2 = xc[:].rearrange("p (t j) -> p t j", t=T)[:, :, 0:16]
                xv2 = xv2.rearrange("p t (l r) -> p t l r", r=2)
                dst = xps[k][:, b * TOKS_B:(b + 1) * TOKS_B]
                with nc.allow_low_precision(reason="2-tap avg-pool emitted in bf16 on purpose"):
                    nc.vector.reduce_sum(dst.rearrange("p (t l) -> p t l", t=T), xv2, AX.X)
            if KSTEP < 2:
                continue
            xptb = [pa.tile([128, TOKT_B], BF16, tag=f"xptb{k}", name=f"xptb{b}_{k}") for k in range(2)]
            for k in range(2):
                nc.scalar.copy(xptb[k][:], xpt[k][:])
            if "xpt" in dbg and b == 0:
                d = dbg_out("xpt", [2, 128, TOKT_B])
                for k in range(2):
                    nc.sync.dma_start(d[k], xpt[k][:])

            # ---- temporal QKV (Q,K as head-pair tiles [64, tok] bf16; V token-major) ----
            qp = [pa.tile([64, TOKT_B], BF16, tag=f"qp{g}", name=f"qp{b}_{g}") for g in range(4)]
            kp = [pa.tile([64, TOKT_B], BF16, tag=f"kp{g}", name=f"kp{b}_{g}") for g in range(4)]
            chunks = [(0, 512), (512, 1024), (1024, TOKT_B)]
            for m in range(4):
                pair = qp if m < 2 else kp
                mh = m % 2
                for (c0, c1) in chunks:
                    ps = pp.tile([128, 512], F32, tag="pbig", name=f"qkps{b}_{m}_{c0}")
                    for k in range(2):
                        nc.tensor.matmul(
                            ps[:, :c1 - c0],
                            wqkvT[k][:, 128 * m:128 * (m + 1)],
                            xptb[k][:, c0:c1],
                            start=(k == 0), stop=(k == 1))
                    nc.scalar.copy(pair[2 * mh][:, c0:c1], ps[0:64, :c1 - c0])
                    nc.scalar.copy(pair[2 * mh + 1][:, c0:c1], ps[64:128, :c1 - c0])
            if KSTEP < 3:
                continue
            vt = pa.tile([128, NT_B * 264], BF16, tag="vt", name=f"vt{b}")
            ones_ap = vt[:].rearrange("p (j h e) -> p j h e", j=NT_B, h=H)[:, :, :, 32]
            nc.vector.memset(ones_ap, 1.0)
            for j in range(NT_B):
                ps = pp.tile([128, 512], F32, tag="pbig", name=f"vps{b}_{j}")
                for k in range(2):
                    nc.tensor.matmul(
                        ps[:81, :256],
                        xptb[k][:, j * LT:(j + 1) * LT],
                        wqkvT[k][:, 512:768],
                        start=(k == 0), stop=(k == 1))
                dst = vt[:81, j * 264:(j + 1) * 264].rearrange("p (h e) -> p h e", h=H)[:, :, 0:32]
                nc.scalar.copy(dst, ps[:81, :256].rearrange("p (h d) -> p h d", h=H))
            if "vt" in dbg and b == 0:
                d = dbg_out("vt", [128, NT_B * 264])
                nc.sync.dma_start(d[:], vt[:])

            if KSTEP < 4:
                continue
            # ---- attention per (j, h): one psum tile per matmul output ----
            otok = pa.tile([128, NT_B * C], BF16, tag="otok", name=f"otok{b}")
            for j in range(NT_B):
                pt = pa2.tile([128, 648], BF16, tag="pt", name=f"pt{b}_{j}")
                for h in range(H):
                    kk = kp[h // 2][32 * (h % 2):32 * (h % 2) + 32, j * LT:(j + 1) * LT]
                    qq = qp[h // 2][32 * (h % 2):32 * (h % 2) + 32, j * LT:(j + 1) * LT]
                    sp = pps.tile([128, 81], F32, tag="sp", name=f"sp{b}_{j}_{h}")
                    nc.tensor.matmul(sp[:81, :81], kk, qq, start=True, stop=True)
                    if KATT < 2:
                        continue
                    nc.scalar.activation(pt[:81, 81 * h:81 * h + 81], sp[:81, :81],
                                         AF.Exp, scale=SCALE)
                if KATT < 3:
                    continue
                rt = pa2.tile([128, 8], F32, tag="rt", name=f"rt{b}_{j}")
                for h in range(H):
                    opt = ppo.tile([128, 40], F32, tag="op", name=f"op{b}_{j}_{h}")
                    nc.tensor.matmul(
                        opt[:81, 0:33],
                        pt[:81, 81 * h:81 * h + 81],
                        vt[:81, j * 264 + 33 * h:j * 264 + 33 * h + 33],
                        start=True, stop=True)
                    if KATT < 4:
                        continue
                    nc.vector.reciprocal(rt[:81, h:h + 1], opt[:81, 32:33])
                    nc.scalar.activation(
                        otok[:81, j * C + 32 * h:j * C + 32 * h + 32],
                        opt[:81, 0:32],
                        AF.Copy, scale=rt[:81, h:h + 1])
            if "otok" in dbg and b == 0:
                d = dbg_out("otok", [128, NT_B * C])
                nc.sync.dma_start(d[:], otok[:])

            if KSTEP < 5:
                continue
            # ---- transpose O to channel-major fp32 ----
            ot = [pa.tile([128, TOKT_B], BF16, tag=f"ot{k}", name=f"ot{b}_{k}") for k in range(2)]
            for j in range(NT_B):
                for k in range(2):
                    pst = ppo.tile([128, 256], BF16, tag="op", name=f"tr{b}_{j}_{k}")
                    nc.tensor.transpose(pst[:128, :81], otok[:81, j * C + 128 * k:j * C + 128 * (k + 1)], idn[:81, :81])
                    nc.scalar.copy(ot[k][:, j * LT:(j + 1) * LT], pst[:128, :81])

            if KSTEP < 6:
                continue
            # ---- proj -> padded, dwconv, stats, store ----
            ypad = [pa.tile([128, NT_B * 83], F32, tag=f"ypad{m}", name=f"ypad{b}_{m}") for m in range(2)]
            for m in range(2):
                zv = ypad[m][:].rearrange("p (j s) -> p j s", j=NT_B)
                nc.vector.memset(zv[:, :, 0], 0.0)
                nc.vector.memset(zv[:, :, 82], 0.0)
            pchunks = [(0, 6), (6, 12), (12, 17)]
            for m in range(2):
                for (j0, j1) in pchunks:
                    ps = pp.tile([128, 512], F32, tag="pbig", name=f"pj{b}_{m}_{j0}")
                    w = (j1 - j0) * LT
                    for k in range(2):
                        nc.tensor.matmul(
                            ps[:, :w],
                            wprojT[k][:, 128 * m:128 * (m + 1)],
                            ot[k][:, j0 * LT:j1 * LT],
                            start=(k == 0), stop=(k == 1))
                    dst = ypad[m][:].rearrange("p (j s) -> p j s", j=NT_B)[:, j0:j1, 1:82]
                    nc.scalar.copy(dst, ps[:, :w].rearrange("p (j t) -> p j t", j=j1 - j0))
            ydw = [pa.tile([128, TOKT_B], F32, tag=f"ydw{m}", name=f"ydw{b}_{m}") for m in range(2)]
            scr = pa.tile([128, TOKT_B], F32, tag="scr", name=f"scr{b}")
            for m in range(2):
                zp = ypad[m][:].rearrange("p (j s) -> p j s", j=NT_B)
                yv = ydw[m][:].rearrange("p (j t) -> p j t", j=NT_B)
                dw = vecs[:, 3 * m:3 * m + 3]
                nc.vector.tensor_scalar_mul(yv, zp[:, :, 1:82], dw[:, 1:2])
                nc.vector.scalar_tensor_tensor(yv, zp[:, :, 0:81], dw[:, 0:1], yv, OP.mult, OP.add)
                nc.vector.scalar_tensor_tensor(yv, zp[:, :, 2:83], dw[:, 2:3], yv, OP.mult, OP.add)
                s1 = pa2.tile([128, 1], F32, tag="s1", name=f"s1{b}_{m}")
                nc.vector.reduce_sum(s1[:], ydw[m][:], AX.X)
                nc.vector.tensor_add(accs[:, m:m + 1], accs[:, m:m + 1], s1[:])
                s2 = pa2.tile([128, 1], F32, tag="s2", name=f"s2{b}_{m}")
                nc.scalar.activation(scr[:], ydw[m][:], AF.Square, accum_out=s2[:])
                nc.vector.tensor_add(accs[:, 2 + m:3 + m], accs[:, 2 + m:3 + m], s2[:])
                ybf = pa2.tile([128, TOKT_B], BF16, tag="ybf", name=f"ybf{b}_{m}")
                nc.scalar.copy(ybf[:], ydw[m][:])
                nc.sync.dma_start(yt_d[m][:, b * TOKT_B:(b + 1) * TOKT_B], ybf[:])
            if "ydw" in dbg and b == 0:
                d = dbg_out("ydw", [2, 128, TOKT_B])
                for m in range(2):
                    nc.sync.dma_start(d[m], ydw[m][:])

    # =================== PHASE A-s: spatial branch to pre-BN ===================
    if PHASES < 2:
        dr_cm.__exit__(None, None, None)
        ex_cm.__exit__(None, None, None)
        return
    with tc.tile_pool(name="sa", bufs=1) as sa, \
         tc.tile_pool(name="sa2", bufs=2) as sa2, \
         tc.tile_pool(name="sp", bufs=4, space="PSUM") as spp:
        for b in range(BSH):
            qs = [sa.tile([128, TOKS_B], BF16, tag=f"qs{m}", name=f"qs{b}_{m}") for m in range(6)]
            schunks = [(0, 486), (486, 972), (972, 1458), (1458, 1944)]
            for m in range(6):
                for (c0, c1) in schunks:
                    ps = spp.tile([128, 512], F32, tag="spbig", name=f"sq{b}_{m}_{c0}")
                    for k in range(2):
                        nc.tensor.matmul(
                            ps[:, :c1 - c0],
                            wqkvS[k][:, 128 * m:128 * (m + 1)],
                            xps[k][:, b * TOKS_B + c0:b * TOKS_B + c1],
                            start=(k == 0), stop=(k == 1))
                    nc.scalar.copy(qs[m][:, c0:c1], ps[:, :c1 - c0])
                nc.sync.dma_start(_qsd(dr, b, m)[:], qs[m][:])
            if "qs" in dbg and b == 0:
                d = dbg_out("qs", [6, 128, TOKS_B])
                for m in range(6):
                    nc.sync.dma_start(d[m], qs[m][:])

            # fold to seq-major [seq, (h,l,d)] / [seq, (h,d,l)] via DRAM
            nrows = [128, NS_B - 128]
            qsm = [sa.tile([128, H * 256], BF16, tag=f"qsm{t}", name=f"qsm{b}_{t}") for t in range(2)]
            ksm = [sa.tile([128, H * 256], BF16, tag=f"ksm{t}", name=f"ksm{b}_{t}") for t in range(2)]
            vsm = [sa.tile([128, H * 256], BF16, tag=f"vsm{t}", name=f"vsm{b}_{t}") for t in range(2)]
            for t in range(2):
                nr = nrows[t]
                for h in range(H):
                    for base, dsts in ((0, qsm), (2, ksm)):
                        src = _qsd(dr, b, base + h // 4)[32 * (h % 4):32 * (h % 4) + 32,
                                                        t * 1024:t * 1024 + 8 * nr]
                        sv = src.rearrange("d (s l) -> s l d", l=LS)
                        dv = dsts[t][:nr, 256 * h:256 * (h + 1)].rearrange("s (l d) -> s l d", l=LS)
                        nc.sync.dma_start(dv, sv)
                    src = _qsd(dr, b, 4 + h // 4)[32 * (h % 4):32 * (h % 4) + 32,
                                                  t * 1024:t * 1024 + 8 * nr]
                    sv = src.rearrange("d (s l) -> s d l", l=LS)
                    dv = vsm[t][:nr, 256 * h:256 * (h + 1)].rearrange("s (d l) -> s d l", d=DH)
                    nc.sync.dma_start(dv, sv)

            # S = QK^T, softmax, O = PV  (DVE broadcast ops, seq-major)
            for t in range(2):
                nr = nrows[t]
                sslab = sa2.tile([128, 512], F32, tag="sslab", name=f"ss{b}_{t}")
                prod = sa2.tile([128, 2048], BF16, tag="prod", name=f"pr{b}_{t}")
                for h in range(H):
                    q3 = qsm[t][:nr, 256 * h:256 * (h + 1)].rearrange("s (l d) -> s l d", l=LS)
                    k3 = ksm[t][:nr, 256 * h:256 * (h + 1)].rearrange("s (l d) -> s l d", l=LS)
                    qb = q3.unsqueeze(2).broadcast_to([nr, LS, LS, DH])
                    kb = k3.unsqueeze(1).broadcast_to([nr, LS, LS, DH])
                    pv = prod[:nr].rearrange("s (q k d) -> s q k d", q=LS, k=LS)
                    nc.vector.tensor_tensor(out=pv, in0=qb, in1=kb, op=OP.mult)
                    nc.vector.reduce_sum(
                        sslab[:nr, 64 * h:64 * (h + 1)].rearrange("s (q k) -> s q k", q=LS),
                        pv, AX.X)
                pslab = sa2.tile([128, 512], BF16, tag="pslab", name=f"pl{b}_{t}")
                nc.scalar.activation(pslab[:nr, :], sslab[:nr, :], AF.Exp, scale=SCALE)
                ssum = sa2.tile([128, 64], F32, tag="ssum", name=f"ssum{b}_{t}")
                nc.vector.reduce_sum(ssum[:nr, :],
                                     pslab[:nr].rearrange("s (hq k) -> s hq k", k=LS), AX.X)
                rcp = sa2.tile([128, 64], F32, tag="rcp", name=f"rcp{b}_{t}")
                nc.vector.reciprocal(rcp[:nr, :], ssum[:nr, :])
                rb = rcp[:nr].unsqueeze(2).broadcast_to([nr, 64, LS])
                p3v = pslab[:nr].rearrange("s (hq k) -> s hq k", k=LS)
                nc.vector.tensor_tensor(out=p3v, in0=p3v, in1=rb, op=OP.mult)
                oslab = sa2.tile([128, 2048], F32, tag="oslab", name=f"os{b}_{t}")
                for h in range(H):
                    p3 = pslab[:nr, 64 * h:64 * (h + 1)].rearrange("s (q k) -> s q k", q=LS)
                    pb = p3.unsqueeze(1).broadcast_to([nr, DH, LS, LS])
                    v3 = vsm[t][:nr, 256 * h:256 * (h + 1)].rearrange("s (d l) -> s d l", d=DH)
                    vb = v3.unsqueeze(2).broadcast_to([nr, DH, LS, LS])
                    pv2 = prod[:nr].rearrange("s (d q k) -> s d q k", d=DH, q=LS)
                    nc.vector.tensor_tensor(out=pv2, in0=pb, in1=vb, op=OP.mult)
                    nc.vector.reduce_sum(
                        oslab[:nr, 256 * h:256 * (h + 1)].rearrange("s (d q) -> s d q", d=DH),
                        pv2, AX.X)
                obf = sa2.tile([128, 2048], BF16, tag="obf", name=f"ob{b}_{t}")
                nc.vector.tensor_copy(obf[:nr, :], oslab[:nr, :])
                for h in range(H):
                    sv = obf[:nr, 256 * h:256 * (h + 1)].rearrange("s (d q) -> s d q", q=LS)
                    dvv = _od2(dr, h // 4)[32 * (h % 4):32 * (h % 4) + 32,
                                           t * 1024:t * 1024 + 8 * nr].rearrange(
                                               "d (s q) -> s d q", q=LS)
                    nc.sync.dma_start(dvv, sv)
            if "pslab" in dbg and b == 0:
                d = dbg_out("pslab", [128, 512])
                nc.sync.dma_start(d[:], pslab[:])

            # load channel-major O^T [256, 1944] from bounce
            ots = [sa.tile([128, TOKS_B], BF16, tag=f"ots{k}", name=f"ots{b}_{k}") for k in range(2)]
            for k in range(2):
                nc.sync.dma_start(ots[k][:], _od2(dr, k)[:])
            if "ots" in dbg and b == 0:
                d = dbg_out("ots", [2, 128, TOKS_B], BF16)
                for k in range(2):
                    nc.sync.dma_start(d[k], ots[k][:])

            # proj -> padded (10 per seq), dwconv over l, stats, store
            yspad = [sa.tile([128, NS_B * 10], F32, tag=f"yspad{m}", name=f"yspad{b}_{m}") for m in range(2)]
            for m in range(2):
                zv = yspad[m][:].rearrange("p (n s) -> p n s", n=NS_B)
                nc.vector.memset(zv[:, :, 0], 0.0)
                nc.vector.memset(zv[:, :, 9], 0.0)
            nchunks = [(0, 61), (61, 122), (122, 183), (183, 243)]
            for m in range(2):
                for (n0, n1) in nchunks:
                    ps = spp.tile([128, 512], F32, tag="spbig", name=f"sp{b}_{m}_{n0}")
                    w = (n1 - n0) * LS
                    for k in range(2):
                        nc.tensor.matmul(
                            ps[:, :w],
                            wprojS[k][:, 128 * m:128 * (m + 1)],
                            ots[k][:, n0 * LS:n1 * LS],
                            start=(k == 0), stop=(k == 1))
                    dst = yspad[m][:].rearrange("p (n s) -> p n s", n=NS_B)[:, n0:n1, 1:9]
                    nc.scalar.copy(dst, ps[:, :w].rearrange("p (n l) -> p n l", n=n1 - n0))
            for m in range(2):
                zp = yspad[m][:].rearrange("p (n s) -> p n s", n=NS_B)
                ydwt = sa.tile([128, TOKS_B], F32, tag=f"ysdw{m}", name=f"ysdw{b}_{m}")
                yv = ydwt[:].rearrange("p (n l) -> p n l", n=NS_B)
                dw = vecs[:, 6 + 3 * m:9 + 3 * m]
                nc.vector.tensor_scalar_mul(yv, zp[:, :, 1:9], dw[:, 1:2])
                nc.vector.scalar_tensor_tensor(yv, zp[:, :, 0:8], dw[:, 0:1], yv, OP.mult, OP.add)
                nc.vector.scalar_tensor_tensor(yv, zp[:, :, 2:10], dw[:, 2:3], yv, OP.mult, OP.add)
                s1 = sa2.tile([128, 1], F32, tag="ss1", name=f"ss1{b}_{m}")
                nc.vector.reduce_sum(s1[:], ydwt[:], AX.X)
                nc.vector.tensor_add(accs[:, 4 + m:5 + m], accs[:, 4 + m:5 + m], s1[:])
                scr2 = sa.tile([128, TOKS_B], F32, tag="sscr", name=f"sscr{b}_{m}")
                s2 = sa2.tile([128, 1], F32, tag="ss2", name=f"ss2{b}_{m}")
                nc.scalar.activation(scr2[:], ydwt[:], AF.Square, accum_out=s2[:])
                nc.vector.tensor_add(accs[:, 6 + m:7 + m], accs[:, 6 + m:7 + m], s2[:])
                ybf = sa2.tile([128, TOKS_B], BF16, tag="ysbf", name=f"ysbf{b}_{m}")
                nc.scalar.copy(ybf[:], ydwt[:])
                nc.sync.dma_start(ys_d[m][:, b * TOKS_B:(b + 1) * TOKS_B], ybf[:])

    # =================== PHASE B: AllReduce stats -> BN coefs ===================
    if PHASES < 3:
        dr_cm.__exit__(None, None, None)
        ex_cm.__exit__(None, None, None)
        return
    bnc = ex.tile([128, 8], F32, name="bnc")  # a_t h0,h1; b_t h0,h1; a_s h0,h1; b_s h0,h1
    with tc.tile_pool(name="pb", bufs=1) as pb:
        cin = dr.tile([128, 8], F32, name="cc_in")
        cout = dr.tile([128, 8], F32, name="cc_out")
        nc.sync.dma_start(cin[:], accs[:])
        nc.gpsimd.collective_compute(
            "AllReduce", OP.add,
            replica_groups=[list(range(N_CORES))],
            ins=[cin.opt()], outs=[cout.opt()])
        gst = pb.tile([128, 8], F32, name="gst")
        nc.sync.dma_start(gst[:], cout[:])
        tmp = pb.tile([128, 8], F32, name="btmp")
        for br, (cnt, sco, gco, bco) in enumerate(
                (((CNT_T), 0, 12, 14), ((CNT_S), 4, 16, 18))):
            for m in range(2):
                mu = pb.tile([128, 1], F32, tag="mu", name=f"mu{br}_{m}")
                nc.scalar.activation(mu[:], gst[:, sco + m:sco + m + 1], AF.Copy, scale=1.0 / cnt)
                m2 = pb.tile([128, 1], F32, tag="m2", name=f"m2{br}_{m}")
                nc.scalar.activation(m2[:], gst[:, sco + 2 + m:sco + 3 + m], AF.Copy, scale=1.0 / cnt)
                # var = m2 - mu^2
                nc.vector.tensor_scalar(out=tmp[:, 0:1], in0=mu[:], scalar1=mu[:],
                                        scalar2=-1.0, op0=OP.mult, op1=OP.mult)
                nc.vector.tensor_add(tmp[:, 1:2], m2[:], tmp[:, 0:1])
                r = pb.tile([128, 1], F32, tag="rr", name=f"r{br}_{m}")
                nc.scalar.activation(tmp[:, 3:4], tmp[:, 1:2], AF.Sqrt, bias=vecs[:, 24:25])
                nc.vector.reciprocal(r[:], tmp[:, 3:4])
                a_col = 4 * br + m
                b_col = 4 * br + 2 + m
                nc.vector.tensor_tensor(out=bnc[:, a_col:a_col + 1],
                                        in0=vecs[:, gco + m:gco + m + 1], in1=r[:], op=OP.mult)
                nc.vector.tensor_tensor(out=tmp[:, 2:3], in0=mu[:],
                                        in1=bnc[:, a_col:a_col + 1], op=OP.mult)
                nc.vector.tensor_sub(bnc[:, b_col:b_col + 1],
                                     vecs[:, bco + m:bco + m + 1], tmp[:, 2:3])
    if "bnc" in dbg:
        d = dbg_out("bnc", [128, 8])
        nc.sync.dma_start(d[:], bnc[:])

    # =================== PHASE C: BN+GELU+pw+interp, fusion MLP ===================
    if PHASES < 4:
        dr_cm.__exit__(None, None, None)
        ex_cm.__exit__(None, None, None)
        return
    lo_s, hi_s, w_s = _interp_lin_coef(LS, J)
    with tc.tile_pool(name="ca", bufs=1) as caq, \
         tc.tile_pool(name="ca2", bufs=2) as ca2, \
         tc.tile_pool(name="cp", bufs=4, space="PSUM") as cp:
        for b in range(BSH):
            comb = [caq.tile([128, T * J], BF16, tag=f"comb{q}", name=f"comb{b}_{q}") for q in range(4)]
            # ---------- temporal tail ----------
            gt = [caq.tile([128, TOKT_B], BF16, tag=f"gt{m}", name=f"gt{b}_{m}") for m in range(2)]
            for m in range(2):
                yl = ca2.tile([128, TOKT_B], BF16, tag="yl", name=f"yl{b}_{m}")
                nc.sync.dma_start(yl[:], yt_d[m][:, b * TOKT_B:(b + 1) * TOKT_B])
                nc.scalar.activation(gt[m][:], yl[:], AF.Gelu,
                                     scale=bnc[:, m:m + 1], bias=bnc[:, 2 + m:3 + m])
            zpad = [caq.tile([128, NT_B * 83], F32, tag=f"zpad{m}", name=f"zpad{b}_{m}") for m in range(2)]
            pchunks = [(0, 6), (6, 12), (12, 17)]
            for m in range(2):
                for (j0, j1) in pchunks:
                    ps = cp.tile([128, 512], F32, tag="cbig", name=f"cpw{b}_{m}_{j0}")
                    w = (j1 - j0) * LT
                    for k in range(2):
                        nc.tensor.matmul(
                            ps[:, :w],
                            wpwT[k][:, 128 * m:128 * (m + 1)],
                            gt[k][:, j0 * LT:j1 * LT],
                            start=(k == 0), stop=(k == 1))
                    dst = zpad[m][:].rearrange("p (j s) -> p j s", j=NT_B)[:, j0:j1, 1:82]
                    nc.scalar.copy(dst, ps[:, :w].rearrange("p (j t) -> p j t", j=j1 - j0))
                zv = zpad[m][:].rearrange("p (j s) -> p j s", j=NT_B)
                nc.vector.tensor_copy(zv[:, :, 0], zv[:, :, 1])
                nc.vector.tensor_copy(zv[:, :, 82], zv[:, :, 81])
                z23 = ca2.tile([128, TOKT_B], F32, tag="z23", name=f"z23{b}_{m}")
                nc.scalar.activation(z23[:].rearrange("p (j t) -> p j t", j=NT_B),
                                     zv[:, :, 1:82], AF.Copy, scale=2.0 / 3.0)
                # out[t=3m+1] = z[m]; out[3m] = z[m-1]/3 + 2z[m]/3; out[3m+2] = z[m+1]/3 + 2z[m]/3
                z23v = z23[:].rearrange("p (j t) -> p j t", j=NT_B)
                dst1 = _interp_dst(comb[m], 1)
                nc.vector.tensor_copy(dst1, zv[:, :, 1:82])
                dst0 = _interp_dst(comb[m], 0)
                nc.vector.scalar_tensor_tensor(dst0, zv[:, :, 0:81], 1.0 / 3.0, z23v, OP.mult, OP.add)
                dst2 = _interp_dst(comb[m], 2)
                nc.vector.scalar_tensor_tensor(dst2, zv[:, :, 2:83], 1.0 / 3.0, z23v, OP.mult, OP.add)
            # ---------- spatial tail ----------
            gs = [caq.tile([128, TOKS_B], BF16, tag=f"gs{m}", name=f"gs{b}_{m}") for m in range(2)]
            for m in range(2):
                yl = ca2.tile([128, TOKS_B], BF16, tag="ysl", name=f"ysl{b}_{m}")
                nc.sync.dma_start(yl[:], ys_d[m][:, b * TOKS_B:(b + 1) * TOKS_B])
                nc.scalar.activation(gs[m][:], yl[:], AF.Gelu,
                                     scale=bnc[:, 4 + m:5 + m], bias=bnc[:, 6 + m:7 + m])
            zs = [caq.tile([128, TOKS_B], F32, tag=f"zs{m}", name=f"zs{b}_{m}") for m in range(2)]
            nchunks = [(0, 61), (61, 122), (122, 183), (183, 243)]
            for m in range(2):
                for (n0, n1) in nchunks:
                    ps = cp.tile([128, 512], F32, tag="cbig", name=f"cps{b}_{m}_{n0}")
                    w = (n1 - n0) * LS
                    for k in range(2):
                        nc.tensor.matmul(
                            ps[:, :w],
                            wpwS[k][:, 128 * m:128 * (m + 1)],
                            gs[k][:, n0 * LS:n1 * LS],
                            start=(k == 0), stop=(k == 1))
                    nc.scalar.copy(zs[m][:, n0 * LS:n1 * LS], ps[:, :w])
                zsv = zs[m][:].rearrange("p (n l) -> p n l", n=NS_B)
                cmv = comb[2 + m][:].rearrange("p (t j) -> p t j", t=T)
                for jj in range(J):
                    lo, hi, w = int(lo_s[jj]), int(hi_s[jj]), float(w_s[jj])
                    if w < 1e-9 or lo == hi:
                        nc.scalar.copy(cmv[:, :, jj], zsv[:, :, lo])
                    else:
                        nc.scalar.activation(cmv[:, :, jj], zsv[:, :, lo], AF.Copy, scale=1.0 - w)
                        nc.vector.scalar_tensor_tensor(cmv[:, :, jj], zsv[:, :, hi], w,
                                                       cmv[:, :, jj], OP.mult, OP.add)
            if "comb" in dbg and b == 0:
                d = dbg_out("comb", [4, 128, T * J], BF16)
                for q in range(4):
                    nc.sync.dma_start(d[q], comb[q][:])

            # ---------- fusion MLP ----------
            g1T = [caq.tile([128, 4144], BF16, tag=f"g1T{k}", name=f"g1T{b}_{k}") for k in range(2)]
            NTOK = T * J  # 4131
            tchunks = [(i * 128, min(NTOK, (i + 1) * 128)) for i in range((NTOK + 127) // 128)]
            for (t0, t1) in tchunks:
                tl = t1 - t0
                ps = cp.tile([128, 256], F32, tag="ch1", name=f"h1{b}_{t0}")
                for q in range(4):
                    nc.tensor.matmul(ps[:tl, :], comb[q][:, t0:t1], fw1T[q][:],
                                     start=(q == 0), stop=(q == 3))
                bnst = ca2.tile([128, 6], F32, tag="bnst", name=f"bnst{b}_{t0}")
                nc.vector.bn_stats(bnst[:tl, :], ps[:tl, :])
                bnag = ca2.tile([128, 2], F32, tag="bnag", name=f"bnag{b}_{t0}")
                nc.vector.bn_aggr(bnag[:tl, :], bnst[:tl, :])
                r = ca2.tile([128, 1], F32, tag="lr", name=f"lr{b}_{t0}")
                sdv = ca2.tile([128, 1], F32, tag="sdv", name=f"sdv{b}_{t0}")
                nc.scalar.activation(sdv[:tl, :], bnag[:tl, 1:2], AF.Sqrt, bias=vecs[:tl, 24:25])
                nc.vector.reciprocal(r[:tl, :], sdv[:tl, :])
                nmr = ca2.tile([128, 1], F32, tag="nmr", name=f"nmr{b}_{t0}")
                nc.vector.tensor_scalar(out=nmr[:tl, :], in0=bnag[:tl, 0:1],
                                        scalar1=r[:tl, :], scalar2=-1.0,
                                        op0=OP.mult, op1=OP.mult)
                g1c = ca2.tile([128, 256], BF16, tag="g1c", name=f"g1c{b}_{t0}")
                nc.scalar.activation(g1c[:tl, :], ps[:tl, :], AF.Gelu,
                                     scale=r[:tl, :], bias=nmr[:tl, :])
                tpad = (tl + 15) // 16 * 16
                for k in range(2):
                    nc.sync.dma_start_transpose(g1T[k][:, t0:t0 + tpad],
                                                g1c[:tpad, 128 * k:128 * (k + 1)])
            h2T = [caq.tile([128, T * J], BF16, tag=f"h2T{m}", name=f"h2T{b}_{m}") for m in range(2)]
            fchunks = [(i * 512, min(NTOK, (i + 1) * 512)) for i in range((NTOK + 511) // 512)]
            for m in range(2):
                for (c0, c1) in fchunks:
                    ps = cp.tile([128, 512], F32, tag="cbig", name=f"h2{b}_{m}_{c0}")
                    for k in range(2):
                        nc.tensor.matmul(ps[:, :c1 - c0], fw2T[k][:, 128 * m:128 * (m + 1)],
                                         g1T[k][:, c0:c1], start=(k == 0), stop=(k == 1))
                    nc.scalar.activation(h2T[m][:, c0:c1], ps[:, :c1 - c0], AF.Identity,
                                         bias=vecs[:, 20 + m:21 + m])
            for m in range(2):
                for (c0, c1) in fchunks:
                    ps = cp.tile([128, 512], F32, tag="cbig", name=f"h3{b}_{m}_{c0}")
                    for k in range(2):
                        nc.tensor.matmul(ps[:, :c1 - c0], pwfT[k][:, 128 * m:128 * (m + 1)],
                                         h2T[k][:, c0:c1], start=(k == 0), stop=(k == 1))
                    ol = ca2.tile([128, 512], F32, tag="ol", name=f"ol{b}_{m}_{c0}")
                    nc.scalar.activation(ol[:, :c1 - c0], ps[:, :c1 - c0], AF.Identity,
                                         bias=vecs[:, 22 + m:23 + m])
                    dst = out_t[b].rearrange("t j c -> c (t j)")[128 * m:128 * (m + 1), c0:c1]
                    nc.sync.dma_start(dst, ol[:, :c1 - c0])

    dr_cm.__exit__(None, None, None)
    ex_cm.__exit__(None, None, None)


def _interp_dst(comb, delta):
    # comb [128, (t j)]: col(t=3mm+delta, j) = mm*51 + delta*17 + j
    v = comb[:].rearrange("p (mm s) -> p mm s", s=51)[:, :, delta * J:(delta + 1) * J]
    return v.transpose([0, 2, 1])  # -> [128, j:17, mm:81] to match z views


_dram_cache = {}


def _qsd(dr, b, m):
    key = ("qsd", m)
    if key not in _dram_cache:
        _dram_cache[key] = dr.tile([128, TOKS_B], BF16, name=f"qsd{m}")
    return _dram_cache[key]


def _od2(dr, k):
    key = ("od2", k)
    if key not in _dram_cache:
        _dram_cache[key] = dr.tile([128, TOKS_B], BF16, name=f"od2_{k}")
    return _dram_cache[key]


def prep_inputs(x, t_qkv, t_proj, t_dw, t_bn_g, t_bn_b, t_pw,
                s_qkv, s_proj, s_dw, s_bn_g, s_bn_b, s_pw,
                f_w1, f_b1, f_ln_g, f_ln_b, f_w2, f_b2, p_w, p_b):
    """Host-side: build per-core in_maps."""
    bf = ml_dtypes.bfloat16
    assert np.all(np.asarray(f_ln_g) == 1.0) and np.all(np.asarray(f_ln_b) == 0.0), \
        "general LayerNorm affine not emitted in this build"
    assert np.all(np.asarray(f_b1) == 0.0), "f_b1 != 0 not emitted in this build"
    common = {
        "wqkv_t": np.ascontiguousarray((np.asarray(t_qkv).T / 3.0).astype(bf)),
        "wqkv_s": np.ascontiguousarray((np.asarray(s_qkv).T / 2.0).astype(bf)),
        "wproj_t": np.ascontiguousarray(np.asarray(t_proj).T.astype(bf)),
        "wproj_s": np.ascontiguousarray(np.asarray(s_proj).T.astype(bf)),
        "wpw_t": np.ascontiguousarray(np.asarray(t_pw).T.astype(bf)),
        "wpw_s": np.ascontiguousarray(np.asarray(s_pw).T.astype(bf)),
        "fw1": np.ascontiguousarray(np.asarray(f_w1).T.astype(bf)),
        "fw2": np.ascontiguousarray(np.asarray(f_w2).T.astype(bf)),
        "pwf": np.ascontiguousarray(np.asarray(p_w).T.astype(bf)),
        "idn": np.eye(128, dtype=bf),
    }
    vecs = np.zeros((128, 26), np.float32)
    vecs[:, 24] = EPS
    tdw = np.asarray(t_dw).reshape(C, 3)
    sdw = np.asarray(s_dw).reshape(C, 3)
    for m in range(2):
        vecs[:, 3 * m:3 * m + 3] = tdw[128 * m:128 * (m + 1)]
        vecs[:, 6 + 3 * m:9 + 3 * m] = sdw[128 * m:128 * (m + 1)]
        vecs[:, 12 + m] = np.asarray(t_bn_g)[128 * m:128 * (m + 1)]
        vecs[:, 14 + m] = np.asarray(t_bn_b)[128 * m:128 * (m + 1)]
        vecs[:, 16 + m] = np.asarray(s_bn_g)[128 * m:128 * (m + 1)]
        vecs[:, 18 + m] = np.asarray(s_bn_b)[128 * m:128 * (m + 1)]
        vecs[:, 20 + m] = np.asarray(f_b2)[128 * m:128 * (m + 1)]
        vecs[:, 22 + m] = np.asarray(p_b)[128 * m:128 * (m + 1)]
    common["vecs"] = vecs
    x = np.asarray(x, dtype=np.float32)
    in_maps = []
    for i in range(N_CORES):
        m = dict(common)
        m["xs"] = np.ascontiguousarray(x[i * BSH:(i + 1) * BSH])
        in_maps.append(m)
    return in_maps


_BUILD_CACHE = {}


def _get_nc(dbg=()):
    key = tuple(sorted(dbg))
    if key not in _BUILD_CACHE:
        _dram_cache.clear()
        nc = bass.Bass(trn_type="TRN2", target_bir_lowering=False, num_devices=N_CORES)
        dbg_outs = build(nc, dbg)
        _BUILD_CACHE[key] = (nc, dbg_outs)
    return _BUILD_CACHE[key]


def run(inputs, dbg=(), trace=False):
    nc, dbg_outs = _get_nc(dbg)
    in_maps = prep_inputs(**inputs)
    res = run_bass_kernel_spmd(nc, in_maps, core_ids=list(range(N_CORES)), trace=trace)
    outs = [r["out"] for r in res.results]
    full = np.concatenate(outs, axis=0)
    return full, res, dbg_outs


def kernel(**inputs) -> np.ndarray:
    full, _, _ = run(inputs)
    return full.astype(np.float32)



# revision 13
# speedup vs baseline: 1.1933x; 1.1933x over previous
"""CeATTForTCPFormer Trainium2 kernel (8 NeuronCores, data-parallel over B).

Contract: kernel(**inputs) takes FULL inputs as in reference.setup_inputs()
and returns the FULL [32, 243, 17, 256] fp32 output. Internally shards B
across 8 cores (4 per core); BN batch stats are combined with one small
AllReduce.
"""
import os
import sys

sys.path.insert(0, "/opt/trn_rl_repo")
sys.path.insert(0, "/opt/trn_rl_repo/concourse")

import numpy as np
import ml_dtypes

import concourse.bass as bass
import concourse.mybir as mybir
import concourse.tile as tile
from concourse.tile_rust import add_dep_helper
from concourse.bass_utils import run_bass_kernel_spmd

F32 = mybir.dt.float32
F32R = mybir.dt.float32r
BF16 = mybir.dt.bfloat16
AF = mybir.ActivationFunctionType
OP = mybir.AluOpType
AX = mybir.AxisListType

N_CORES = 8
B, T, J, C = 32, 243, 17, 256
BSH = B // N_CORES          # 4 batch elems per core
H, DH = 8, 32
LT = 81                     # temporal pooled length (243/3)
LS = 8                      # spatial pooled length (17//2)
NT_B = J                    # temporal seqs per batch elem
TOKT_B = J * LT             # 1377 temporal tokens per b
NS_B = T                    # spatial seqs per b (243)
TOKS_B = T * LS             # 1944 spatial tokens per b
CNT_T = float(B * J * LT)   # global BN count temporal = 44064
CNT_S = float(B * T * LS)   # spatial N = B*T, L = 8 -> 62208
SCALE = 1.0 / np.sqrt(DH)
EPS = 1e-5

# walrus in this container accepts at most 1 sync-wait command per
# instruction; Tile's tail drain carries one wait per logical processor.
MAX_WAITS = 1


def _split_excess_waits(nc):
    ctr = 0
    for f in nc.m.functions:
        for bb in f.blocks:
            new_insts, changed = [], False
            for inst in bb.instructions:
                si = inst.sync_info
                if si is not None and si.on_wait is not None and len(si.on_wait) > MAX_WAITS:
                    waits = list(si.on_wait)
                    upd = list(si.on_update or [])
                    rest, keep = waits[:-MAX_WAITS], waits[-MAX_WAITS:]
                    for w in rest:
                        nop = mybir.InstNoOp(name=f"waitsplit_{ctr}", ins=[], outs=[])
                        ctr += 1
                        nop.engine = inst.engine
                        nop.sync_info = mybir.SyncInfo(on_wait=[w], on_update=[])
                        new_insts.append(nop)
                    inst.sync_info = mybir.SyncInfo(on_wait=keep, on_update=upd)
                    changed = True
                new_insts.append(inst)
            if changed:
                bb.instructions = new_insts


def _interp_lin_coef(L, out_len):
    scale = L / out_len
    coords = (np.arange(out_len) + 0.5) * scale - 0.5
    coords = np.clip(coords, 0.0, L - 1)
    lo = np.floor(coords).astype(np.int32)
    hi = np.minimum(lo + 1, L - 1)
    w = (coords - lo).astype(np.float64)
    return lo, hi, w


def build(nc, dbg=()):
    """Emit the whole per-core program. Returns list of debug output names."""
    dbg = set(dbg)
    dbg_outs = []

    x_t = nc.dram_tensor("xs", [BSH, T, J, C], F32, kind="ExternalInput").ap()
    wqkv_t = nc.dram_tensor("wqkv_t", [C, 3 * C], BF16, kind="ExternalInput").ap()
    wqkv_s = nc.dram_tensor("wqkv_s", [C, 3 * C], BF16, kind="ExternalInput").ap()
    wproj_t = nc.dram_tensor("wproj_t", [C, C], BF16, kind="ExternalInput").ap()
    wproj_s = nc.dram_tensor("wproj_s", [C, C], BF16, kind="ExternalInput").ap()
    wpw_t = nc.dram_tensor("wpw_t", [C, C], BF16, kind="ExternalInput").ap()
    wpw_s = nc.dram_tensor("wpw_s", [C, C], BF16, kind="ExternalInput").ap()
    fw1_t = nc.dram_tensor("fw1", [2 * C, C], BF16, kind="ExternalInput").ap()
    fw2_t = nc.dram_tensor("fw2", [C, C], BF16, kind="ExternalInput").ap()
    pwf_t = nc.dram_tensor("pwf", [C, C], BF16, kind="ExternalInput").ap()
    idn_t = nc.dram_tensor("idn", [128, 128], BF16, kind="ExternalInput").ap()
    # vecs columns: 0-2 t_dw half0, 3-5 t_dw half1, 6-8 s_dw h0, 9-11 s_dw h1,
    # 12,13 t_bn_g h0/h1, 14,15 t_bn_b, 16,17 s_bn_g, 18,19 s_bn_b,
    # 20,21 f_b2, 22,23 p_b
    vecs_t = nc.dram_tensor("vecs", [128, 26], F32, kind="ExternalInput").ap()
    out_t = nc.dram_tensor("out", [BSH, T, J, C], F32, kind="ExternalOutput").ap()

    def dbg_out(name, shape, dtype=F32):
        ap = nc.dram_tensor("dbg_" + name, shape, dtype, kind="ExternalOutput").ap()
        dbg_outs.append("dbg_" + name)
        return ap

    tc = TCFix(nc)
    with tc:
        _build_body(nc, tc, locals(), dbg, dbg_out)
    _split_excess_waits(nc)
    return dbg_outs


class TCFix(tile.TileContext):
    pass  # tail-drain waits are split post-hoc by _split_excess_waits


PHASES = int(os.environ.get("KPHASES", "4"))


KSTEP = int(os.environ.get("KSTEP", "99"))
KATT = int(os.environ.get("KATT", "99"))


def _build_body(nc, tc, ctx, dbg, dbg_out):
    x_t = ctx["x_t"]; out_t = ctx["out_t"]; vecs_t = ctx["vecs_t"]
    wqkv_t = ctx["wqkv_t"]; wqkv_s = ctx["wqkv_s"]
    wproj_t = ctx["wproj_t"]; wproj_s = ctx["wproj_s"]
    wpw_t = ctx["wpw_t"]; wpw_s = ctx["wpw_s"]
    fw1_t = ctx["fw1_t"]; fw2_t = ctx["fw2_t"]; pwf_t = ctx["pwf_t"]
    idn_t = ctx["idn_t"]

    ex_cm = tc.tile_pool(name="ex", bufs=1)       # persistent: weights, stats
    ex = ex_cm.__enter__()
    dr_cm = tc.tile_pool(name="dr", bufs=1, space="DRAM")
    dr = dr_cm.__enter__()

    # ---- persistent weight tiles ----
    wqkvT = [ex.tile([128, 3 * C], BF16, name=f"wqkvT{k}") for k in range(2)]
    for k in range(2):
        nc.sync.dma_start(wqkvT[k][:], wqkv_t[128 * k:128 * (k + 1), :])
    wqkvS = [ex.tile([128, 3 * C], BF16, name=f"wqkvS{k}") for k in range(2)]
    for k in range(2):
        nc.sync.dma_start(wqkvS[k][:], wqkv_s[128 * k:128 * (k + 1), :])
    wprojT = [ex.tile([128, C], BF16, name=f"wprojT{k}") for k in range(2)]
    for k in range(2):
        nc.sync.dma_start(wprojT[k][:], wproj_t[128 * k:128 * (k + 1), :])
    wprojS = [ex.tile([128, C], BF16, name=f"wprojS{k}") for k in range(2)]
    for k in range(2):
        nc.sync.dma_start(wprojS[k][:], wproj_s[128 * k:128 * (k + 1), :])
    wpwT = [ex.tile([128, C], BF16, name=f"wpwT{k}") for k in range(2)]
    for k in range(2):
        nc.sync.dma_start(wpwT[k][:], wpw_t[128 * k:128 * (k + 1), :])
    wpwS = [ex.tile([128, C], BF16, name=f"wpwS{k}") for k in range(2)]
    for k in range(2):
        nc.sync.dma_start(wpwS[k][:], wpw_s[128 * k:128 * (k + 1), :])
    fw1T = [ex.tile([128, C], BF16, name=f"fw1T{k}") for k in range(4)]
    for k in range(4):
        nc.sync.dma_start(fw1T[k][:], fw1_t[128 * k:128 * (k + 1), :])
    fw2T = [ex.tile([128, C], BF16, name=f"fw2T{k}") for k in range(2)]
    for k in range(2):
        nc.sync.dma_start(fw2T[k][:], fw2_t[128 * k:128 * (k + 1), :])
    pwfT = [ex.tile([128, C], BF16, name=f"pwfT{k}") for k in range(2)]
    for k in range(2):
        nc.sync.dma_start(pwfT[k][:], pwf_t[128 * k:128 * (k + 1), :])
    idn = ex.tile([128, 128], BF16, name="idn")
    nc.sync.dma_start(idn[:], idn_t[:])
    vecs = ex.tile([128, 26], F32, name="vecs")
    nc.sync.dma_start(vecs[:], vecs_t[:])

    # spatial pooled input, bf16, built during temporal loop (uses X residency)
    xps = [ex.tile([128, BSH * TOKS_B], BF16, name=f"xps{k}") for k in range(2)]
    # BN partial accumulators: [t_sum h0,h1, t_sq h0,h1, s_sum h0,h1, s_sq h0,h1]
    accs = ex.tile([128, 8], F32, name="accs")
    nc.vector.memset(accs[:], 0.0)
    # temporal/spatial pre-BN activations parked in DRAM (bf16)
    yt_d = [dr.tile([128, BSH * TOKT_B], BF16, name=f"yt_d{k}") for k in range(2)]
    ys_d = [dr.tile([128, BSH * TOKS_B], BF16, name=f"ys_d{k}") for k in range(2)]

    # =================== PHASE A-t: temporal branch to pre-BN ===================
    with tc.tile_pool(name="pa", bufs=1) as pa, \
         tc.tile_pool(name="pa2", bufs=2) as pa2, \
         tc.tile_pool(name="pp", bufs=2, space="PSUM") as pp, \
         tc.tile_pool(name="pps", bufs=2, space="PSUM") as pps, \
         tc.tile_pool(name="ppo", bufs=2, space="PSUM") as ppo:
        for b in range(BSH):
            xpt = [pa.tile([128, TOKT_B], BF16, tag=f"xpt{k}", name=f"xpt{b}_{k}") for k in range(2)]
            for k in range(2):
                xc = pa2.tile([128, T * J], F32, tag="xc", name=f"xc{b}_{k}")
                src = x_t[b].rearrange("t j c -> c (t j)")[128 * k:128 * (k + 1), :]
                nc.sync.dma_start(xc[:], src)
                # temporal pool: out[c, j*81+m] = sum_r x[c, (3m+r)*17+j]
                xv = xc[:].rearrange("p (t j) -> p j t", j=J)  # [128, j, 243]
                xv = xv.rearrange("p j (m r) -> p j m r", r=3)
                with nc.allow_low_precision(reason="3-tap avg-pool emitted in bf16 on purpose"):
                    nc.vector.reduce_sum(xpt[k][:].rearrange("p (j m) -> p j m", j=J), xv, AX.X)
                # spatial pool: out[c, b*1944 + t*8 + l] = sum_r x[c, t*17 + 2l+r]
                xv2 = xc[:].rearrange("p (t j) -> p t j", t=T)[:, :, 0:16]
                xv2 = xv2.rearrange("p t (l r) -> p t l r", r=2)
                dst = xps[k][:, b * TOKS_B:(b + 1) * TOKS_B]
                with nc.allow_low_precision(reason="2-tap avg-pool emitted in bf16 on purpose"):
                    nc.vector.reduce_sum(dst.rearrange("p (t l) -> p t l", t=T), xv2, AX.X)
            if KSTEP < 2:
                continue
            xptb = xpt
            if "xpt" in dbg and b == 0:
                d = dbg_out("xpt", [2, 128, TOKT_B])
                for k in range(2):
                    nc.sync.dma_start(d[k], xpt[k][:])

            # ---- temporal QKV (Q,K as head-pair tiles [64, tok] bf16; V token-major) ----
            qp = [pa.tile([64, TOKT_B], BF16, tag=f"qp{g}", name=f"qp{b}_{g}") for g in range(4)]
            kp = [pa.tile([64, TOKT_B], BF16, tag=f"kp{g}", name=f"kp{b}_{g}") for g in range(4)]
            chunks = [(0, 512), (512, 1024), (1024, TOKT_B)]
            for m in range(4):
                pair = qp if m < 2 else kp
                mh = m % 2
                for (c0, c1) in chunks:
                    ps = pp.tile([128, 512], F32, tag="pbig", name=f"qkps{b}_{m}_{c0}")
                    for k in range(2):
                        nc.tensor.matmul(
                            ps[:, :c1 - c0],
                            wqkvT[k][:, 128 * m:128 * (m + 1)],
                            xptb[k][:, c0:c1],
                            start=(k == 0), stop=(k == 1))
                    nc.scalar.copy(pair[2 * mh][:, c0:c1], ps[0:64, :c1 - c0])
                    nc.scalar.copy(pair[2 * mh + 1][:, c0:c1], ps[64:128, :c1 - c0])
            if KSTEP < 3:
                continue
            vt = pa.tile([128, NT_B * 264], BF16, tag="vt", name=f"vt{b}")
            ones_ap = vt[:].rearrange("p (j h e) -> p j h e", j=NT_B, h=H)[:, :, :, 32]
            nc.vector.memset(ones_ap, 1.0)
            for j in range(NT_B):
                ps = pp.tile([128, 512], F32, tag="pbig", name=f"vps{b}_{j}")
                for k in range(2):
                    nc.tensor.matmul(
                        ps[:81, :256],
                        xptb[k][:, j * LT:(j + 1) * LT],
                        wqkvT[k][:, 512:768],
                        start=(k == 0), stop=(k == 1))
                dst = vt[:81, j * 264:(j + 1) * 264].rearrange("p (h e) -> p h e", h=H)[:, :, 0:32]
                nc.scalar.copy(dst, ps[:81, :256].rearrange("p (h d) -> p h d", h=H))
            if "vt" in dbg and b == 0:
                d = dbg_out("vt", [128, NT_B * 264])
                nc.sync.dma_start(d[:], vt[:])

            if KSTEP < 4:
                continue
            # ---- attention per (j, 4-head group) ----
            otok = pa.tile([128, NT_B * C], BF16, tag="otok", name=f"otok{b}")
            for j in range(NT_B):
                pt = pa2.tile([128, 648], BF16, tag="pt", name=f"pt{b}_{j}")
                for g in range(2):
                    spb = pps.tile([128, 512], F32, tag="sp", name=f"sp{b}_{j}_{g}")
                    for hh in range(4):
                        h = 4 * g + hh
                        kk = kp[h // 2][32 * (h % 2):32 * (h % 2) + 32, j * LT:(j + 1) * LT]
                        qq = qp[h // 2][32 * (h % 2):32 * (h % 2) + 32, j * LT:(j + 1) * LT]
                        nc.tensor.matmul(spb[:81, 81 * hh:81 * hh + 81], kk, qq,
                                         start=True, stop=True)
                    if KATT < 2:
                        continue
                    nc.scalar.activation(pt[:81, 324 * g:324 * g + 324],
                                         spb[:81, :324], AF.Exp, scale=SCALE)
                if KATT < 3:
                    continue
                rt = pa2.tile([128, 8], F32, tag="rt", name=f"rt{b}_{j}")
                for g in range(2):
                    opt = ppo.tile([128, 136], F32, tag="op", name=f"op{b}_{j}_{g}")
                    for hh in range(4):
                        h = 4 * g + hh
                        nc.tensor.matmul(
                            opt[:81, 33 * hh:33 * hh + 33],
                            pt[:81, 81 * h:81 * h + 81],
                            vt[:81, j * 264 + 33 * h:j * 264 + 33 * h + 33],
                            start=True, stop=True)
                    if KATT < 4:
                        continue
                    ov = opt[:81, 0:132].rearrange("p (h e) -> p h e", h=4)
                    nc.vector.reciprocal(rt[:81, 4 * g:4 * g + 4], ov[:, :, 32])
                    dst = otok[:81, j * C + 128 * g:j * C + 128 * (g + 1)].rearrange(
                        "p (h d) -> p h d", h=4)
                    rb = rt[:81, 4 * g:4 * g + 4].unsqueeze(2).broadcast_to([81, 4, 32])
                    nc.vector.tensor_tensor(out=dst, in0=ov[:, :, 0:32], in1=rb,
                                            op=OP.mult)
            if "otok" in dbg and b == 0:
                d = dbg_out("otok", [128, NT_B * C])
                nc.sync.dma_start(d[:], otok[:])

            if KSTEP < 5:
                continue
            # ---- transpose O to channel-major fp32 ----
            ot = [pa.tile([128, TOKT_B], BF16, tag=f"ot{k}", name=f"ot{b}_{k}") for k in range(2)]
            for j in range(NT_B):
                for k in range(2):
                    pst = ppo.tile([128, 256], BF16, tag="tr", name=f"tr{b}_{j}_{k}")
                    nc.tensor.transpose(pst[:128, :81], otok[:81, j * C + 128 * k:j * C + 128 * (k + 1)], idn[:81, :81])
                    nc.scalar.copy(ot[k][:, j * LT:(j + 1) * LT], pst[:128, :81])

            if KSTEP < 6:
                continue
            # ---- proj -> padded, dwconv, stats, store ----
            ypad = [pa.tile([128, NT_B * 83], F32, tag=f"ypad{m}", name=f"ypad{b}_{m}") for m in range(2)]
            for m in range(2):
                zv = ypad[m][:].rearrange("p (j s) -> p j s", j=NT_B)
                nc.vector.memset(zv[:, :, 0], 0.0)
                nc.vector.memset(zv[:, :, 82], 0.0)
            pchunks = [(0, 6), (6, 12), (12, 17)]
            for m in range(2):
                for (j0, j1) in pchunks:
                    ps = pp.tile([128, 512], F32, tag="pbig", name=f"pj{b}_{m}_{j0}")
                    w = (j1 - j0) * LT
                    for k in range(2):
                        nc.tensor.matmul(
                            ps[:, :w],
                            wprojT[k][:, 128 * m:128 * (m + 1)],
                            ot[k][:, j0 * LT:j1 * LT],
                            start=(k == 0), stop=(k == 1))
                    dst = ypad[m][:].rearrange("p (j s) -> p j s", j=NT_B)[:, j0:j1, 1:82]
                    nc.scalar.copy(dst, ps[:, :w].rearrange("p (j t) -> p j t", j=j1 - j0))
            ydw = [pa.tile([128, TOKT_B], BF16, tag=f"ydw{m}", name=f"ydw{b}_{m}") for m in range(2)]
            scr = pa.tile([128, TOKT_B], BF16, tag="scr", name=f"scr{b}")
            for m in range(2):
                zp = ypad[m][:].rearrange("p (j s) -> p j s", j=NT_B)
                yv = ydw[m][:].rearrange("p (j t) -> p j t", j=NT_B)
                dw = vecs[:, 3 * m:3 * m + 3]
                nc.gpsimd.tensor_scalar_mul(yv, zp[:, :, 1:82], dw[:, 1:2])
                nc.gpsimd.scalar_tensor_tensor(yv, zp[:, :, 0:81], dw[:, 0:1], yv, OP.mult, OP.add)
                nc.gpsimd.scalar_tensor_tensor(yv, zp[:, :, 2:83], dw[:, 2:3], yv, OP.mult, OP.add)
                s1 = pa2.tile([128, 1], F32, tag="s1", name=f"s1{b}_{m}")
                nc.vector.reduce_sum(s1[:], ydw[m][:], AX.X)
                nc.vector.tensor_add(accs[:, m:m + 1], accs[:, m:m + 1], s1[:])
                s2 = pa2.tile([128, 1], F32, tag="s2", name=f"s2{b}_{m}")
                nc.vector.tensor_tensor_reduce(out=scr[:], in0=ydw[m][:], in1=ydw[m][:],
                                               scale=1.0, scalar=0.0,
                                               op0=OP.mult, op1=OP.add, accum_out=s2[:])
                nc.vector.tensor_add(accs[:, 2 + m:3 + m], accs[:, 2 + m:3 + m], s2[:])
                nc.sync.dma_start(yt_d[m][:, b * TOKT_B:(b + 1) * TOKT_B], ydw[m][:])
            if "ydw" in dbg and b == 0:
                d = dbg_out("ydw", [2, 128, TOKT_B], BF16)
                for m in range(2):
                    nc.sync.dma_start(d[m], ydw[m][:])

    # =================== PHASE A-s: spatial branch to pre-BN ===================
    if PHASES < 2:
        dr_cm.__exit__(None, None, None)
        ex_cm.__exit__(None, None, None)
        return
    with tc.tile_pool(name="sa", bufs=1) as sa, \
         tc.tile_pool(name="sa2", bufs=2) as sa2, \
         tc.tile_pool(name="sp", bufs=4, space="PSUM") as spp:
        for b in range(BSH):
            qs = [sa.tile([128, TOKS_B], BF16, tag=f"qs{m}", name=f"qs{b}_{m}") for m in range(6)]
            schunks = [(0, 486), (486, 972), (972, 1458), (1458, 1944)]
            for m in range(6):
                for (c0, c1) in schunks:
                    ps = spp.tile([128, 512], F32, tag="spbig", name=f"sq{b}_{m}_{c0}")
                    for k in range(2):
                        nc.tensor.matmul(
                            ps[:, :c1 - c0],
                            wqkvS[k][:, 128 * m:128 * (m + 1)],
                            xps[k][:, b * TOKS_B + c0:b * TOKS_B + c1],
                            start=(k == 0), stop=(k == 1))
                    nc.scalar.copy(qs[m][:, c0:c1], ps[:, :c1 - c0])
                nc.sync.dma_start(_qsd(dr, b, m)[:], qs[m][:])
            if "qs" in dbg and b == 0:
                d = dbg_out("qs", [6, 128, TOKS_B])
                for m in range(6):
                    nc.sync.dma_start(d[m], qs[m][:])

            # fold to seq-major [seq, (h,l,d)] / [seq, (h,d,l)] via DRAM
            nrows = [128, NS_B - 128]
            qsm = [sa.tile([128, H * 256], BF16, tag=f"qsm{t}", name=f"qsm{b}_{t}") for t in range(2)]
            ksm = [sa.tile([128, H * 256], BF16, tag=f"ksm{t}", name=f"ksm{b}_{t}") for t in range(2)]
            vsm = [sa.tile([128, H * 256], BF16, tag=f"vsm{t}", name=f"vsm{b}_{t}") for t in range(2)]
            for t in range(2):
                nr = nrows[t]
                for h in range(H):
                    for base, dsts in ((0, qsm), (2, ksm)):
                        src = _qsd(dr, b, base + h // 4)[32 * (h % 4):32 * (h % 4) + 32,
                                                        t * 1024:t * 1024 + 8 * nr]
                        sv = src.rearrange("d (s l) -> s l d", l=LS)
                        dv = dsts[t][:nr, 256 * h:256 * (h + 1)].rearrange("s (l d) -> s l d", l=LS)
                        nc.sync.dma_start(dv, sv)
                    src = _qsd(dr, b, 4 + h // 4)[32 * (h % 4):32 * (h % 4) + 32,
                                                  t * 1024:t * 1024 + 8 * nr]
                    sv = src.rearrange("d (s l) -> s d l", l=LS)
                    dv = vsm[t][:nr, 256 * h:256 * (h + 1)].rearrange("s (d l) -> s d l", d=DH)
                    nc.sync.dma_start(dv, sv)

            # S = QK^T, softmax, O = PV  (DVE broadcast ops, seq-major)
            for t in range(2):
                nr = nrows[t]
                sslab = sa2.tile([128, 512], BF16, tag="sslab", name=f"ss{b}_{t}")
                prod = sa2.tile([128, 2048], BF16, tag="prod", name=f"pr{b}_{t}")
                for h in range(H):
                    q3 = qsm[t][:nr, 256 * h:256 * (h + 1)].rearrange("s (l d) -> s l d", l=LS)
                    k3 = ksm[t][:nr, 256 * h:256 * (h + 1)].rearrange("s (l d) -> s l d", l=LS)
                    qb = q3.unsqueeze(2).broadcast_to([nr, LS, LS, DH])
                    kb = k3.unsqueeze(1).broadcast_to([nr, LS, LS, DH])
                    pv = prod[:nr].rearrange("s (q k d) -> s q k d", q=LS, k=LS)
                    nc.vector.tensor_tensor(out=pv, in0=qb, in1=kb, op=OP.mult)
                    with nc.allow_low_precision(reason="attn scores in bf16 on purpose"):
                        nc.vector.reduce_sum(
                            sslab[:nr, 64 * h:64 * (h + 1)].rearrange("s (q k) -> s q k", q=LS),
                            pv, AX.X)
                pslab = sa2.tile([128, 512], BF16, tag="pslab", name=f"pl{b}_{t}")
                nc.scalar.activation(pslab[:nr, :], sslab[:nr, :], AF.Exp, scale=SCALE)
                ssum = sa2.tile([128, 64], F32, tag="ssum", name=f"ssum{b}_{t}")
                nc.vector.reduce_sum(ssum[:nr, :],
                                     pslab[:nr].rearrange("s (hq k) -> s hq k", k=LS), AX.X)
                rcp = sa2.tile([128, 64], F32, tag="rcp", name=f"rcp{b}_{t}")
                nc.vector.reciprocal(rcp[:nr, :], ssum[:nr, :])
                rb = rcp[:nr].unsqueeze(2).broadcast_to([nr, 64, LS])
                p3v = pslab[:nr].rearrange("s (hq k) -> s hq k", k=LS)
                nc.vector.tensor_tensor(out=p3v, in0=p3v, in1=rb, op=OP.mult)
                oslab = sa2.tile([128, 2048], BF16, tag="oslab", name=f"os{b}_{t}")
                for h in range(H):
                    p3 = pslab[:nr, 64 * h:64 * (h + 1)].rearrange("s (q k) -> s q k", q=LS)
                    pb = p3.unsqueeze(1).broadcast_to([nr, DH, LS, LS])
                    v3 = vsm[t][:nr, 256 * h:256 * (h + 1)].rearrange("s (d l) -> s d l", d=DH)
                    vb = v3.unsqueeze(2).broadcast_to([nr, DH, LS, LS])
                    pv2 = prod[:nr].rearrange("s (d q k) -> s d q k", d=DH, q=LS)
                    nc.vector.tensor_tensor(out=pv2, in0=pb, in1=vb, op=OP.mult)
                    with nc.allow_low_precision(reason="attn out in bf16 on purpose"):
                        nc.vector.reduce_sum(
                            oslab[:nr, 256 * h:256 * (h + 1)].rearrange("s (d q) -> s d q", d=DH),
                            pv2, AX.X)
                for h in range(H):
                    sv = oslab[:nr, 256 * h:256 * (h + 1)].rearrange("s (d q) -> s d q", q=LS)
                    dvv = _od2(dr, h // 4)[32 * (h % 4):32 * (h % 4) + 32,
                                           t * 1024:t * 1024 + 8 * nr].rearrange(
                                               "d (s q) -> s d q", q=LS)
                    nc.sync.dma_start(dvv, sv)
            if "pslab" in dbg and b == 0:
                d = dbg_out("pslab", [128, 512])
                nc.sync.dma_start(d[:], pslab[:])

            # load channel-major O^T [256, 1944] from bounce
            ots = [sa.tile([128, TOKS_B], BF16, tag=f"ots{k}", name=f"ots{b}_{k}") for k in range(2)]
            for k in range(2):
                nc.sync.dma_start(ots[k][:], _od2(dr, k)[:])
            if "ots" in dbg and b == 0:
                d = dbg_out("ots", [2, 128, TOKS_B], BF16)
                for k in range(2):
                    nc.sync.dma_start(d[k], ots[k][:])

            # proj -> padded (10 per seq), dwconv over l, stats, store
            yspad = [sa.tile([128, NS_B * 10], F32, tag=f"yspad{m}", name=f"yspad{b}_{m}") for m in range(2)]
            for m in range(2):
                zv = yspad[m][:].rearrange("p (n s) -> p n s", n=NS_B)
                nc.vector.memset(zv[:, :, 0], 0.0)
                nc.vector.memset(zv[:, :, 9], 0.0)
            nchunks = [(0, 61), (61, 122), (122, 183), (183, 243)]
            for m in range(2):
                for (n0, n1) in nchunks:
                    ps = spp.tile([128, 512], F32, tag="spbig", name=f"sp{b}_{m}_{n0}")
                    w = (n1 - n0) * LS
                    for k in range(2):
                        nc.tensor.matmul(
                            ps[:, :w],
                            wprojS[k][:, 128 * m:128 * (m + 1)],
                            ots[k][:, n0 * LS:n1 * LS],
                            start=(k == 0), stop=(k == 1))
                    dst = yspad[m][:].rearrange("p (n s) -> p n s", n=NS_B)[:, n0:n1, 1:9]
                    nc.scalar.copy(dst, ps[:, :w].rearrange("p (n l) -> p n l", n=n1 - n0))
            for m in range(2):
                zp = yspad[m][:].rearrange("p (n s) -> p n s", n=NS_B)
                ydwt = sa.tile([128, TOKS_B], BF16, tag=f"ysdw{m}", name=f"ysdw{b}_{m}")
                yv = ydwt[:].rearrange("p (n l) -> p n l", n=NS_B)
                dw = vecs[:, 6 + 3 * m:9 + 3 * m]
                nc.gpsimd.tensor_scalar_mul(yv, zp[:, :, 1:9], dw[:, 1:2])
                nc.gpsimd.scalar_tensor_tensor(yv, zp[:, :, 0:8], dw[:, 0:1], yv, OP.mult, OP.add)
                nc.gpsimd.scalar_tensor_tensor(yv, zp[:, :, 2:10], dw[:, 2:3], yv, OP.mult, OP.add)
                s1 = sa2.tile([128, 1], F32, tag="ss1", name=f"ss1{b}_{m}")
                nc.vector.reduce_sum(s1[:], ydwt[:], AX.X)
                nc.vector.tensor_add(accs[:, 4 + m:5 + m], accs[:, 4 + m:5 + m], s1[:])
                scr2 = sa.tile([128, TOKS_B], BF16, tag="sscr", name=f"sscr{b}_{m}")
                s2 = sa2.tile([128, 1], F32, tag="ss2", name=f"ss2{b}_{m}")
                nc.vector.tensor_tensor_reduce(out=scr2[:], in0=ydwt[:], in1=ydwt[:],
                                               scale=1.0, scalar=0.0,
                                               op0=OP.mult, op1=OP.add, accum_out=s2[:])
                nc.vector.tensor_add(accs[:, 6 + m:7 + m], accs[:, 6 + m:7 + m], s2[:])
                nc.sync.dma_start(ys_d[m][:, b * TOKS_B:(b + 1) * TOKS_B], ydwt[:])

    # =================== PHASE B: AllReduce stats -> BN coefs ===================
    if PHASES < 3:
        dr_cm.__exit__(None, None, None)
        ex_cm.__exit__(None, None, None)
        return
    bnc = ex.tile([128, 8], F32, name="bnc")  # a_t h0,h1; b_t h0,h1; a_s h0,h1; b_s h0,h1
    with tc.tile_pool(name="pb", bufs=1) as pb:
        cin = dr.tile([128, 8], F32, name="cc_in")
        cout = dr.tile([128, 8], F32, name="cc_out")
        nc.sync.dma_start(cin[:], accs[:])
        nc.gpsimd.collective_compute(
            "AllReduce", OP.add,
            replica_groups=[list(range(N_CORES))],
            ins=[cin.opt()], outs=[cout.opt()])
        gst = pb.tile([128, 8], F32, name="gst")
        nc.sync.dma_start(gst[:], cout[:])
        tmp = pb.tile([128, 8], F32, name="btmp")
        for br, (cnt, sco, gco, bco) in enumerate(
                (((CNT_T), 0, 12, 14), ((CNT_S), 4, 16, 18))):
            for m in range(2):
                mu = pb.tile([128, 1], F32, tag="mu", name=f"mu{br}_{m}")
                nc.scalar.activation(mu[:], gst[:, sco + m:sco + m + 1], AF.Copy, scale=1.0 / cnt)
                m2 = pb.tile([128, 1], F32, tag="m2", name=f"m2{br}_{m}")
                nc.scalar.activation(m2[:], gst[:, sco + 2 + m:sco + 3 + m], AF.Copy, scale=1.0 / cnt)
                # var = m2 - mu^2
                nc.vector.tensor_scalar(out=tmp[:, 0:1], in0=mu[:], scalar1=mu[:],
                                        scalar2=-1.0, op0=OP.mult, op1=OP.mult)
                nc.vector.tensor_add(tmp[:, 1:2], m2[:], tmp[:, 0:1])
                r = pb.tile([128, 1], F32, tag="rr", name=f"r{br}_{m}")
                nc.scalar.activation(tmp[:, 3:4], tmp[:, 1:2], AF.Sqrt, bias=vecs[:, 24:25])
                nc.vector.reciprocal(r[:], tmp[:, 3:4])
                a_col = 4 * br + m
                b_col = 4 * br + 2 + m
                nc.vector.tensor_tensor(out=bnc[:, a_col:a_col + 1],
                                        in0=vecs[:, gco + m:gco + m + 1], in1=r[:], op=OP.mult)
                nc.vector.tensor_tensor(out=tmp[:, 2:3], in0=mu[:],
                                        in1=bnc[:, a_col:a_col + 1], op=OP.mult)
                nc.vector.tensor_sub(bnc[:, b_col:b_col + 1],
                                     vecs[:, bco + m:bco + m + 1], tmp[:, 2:3])
    if "bnc" in dbg:
        d = dbg_out("bnc", [128, 8])
        nc.sync.dma_start(d[:], bnc[:])

    # =================== PHASE C: BN+GELU+pw+interp, fusion MLP ===================
    if PHASES < 4:
        dr_cm.__exit__(None, None, None)
        ex_cm.__exit__(None, None, None)
        return
    lo_s, hi_s, w_s = _interp_lin_coef(LS, J)
    with tc.tile_pool(name="ca", bufs=1) as caq, \
         tc.tile_pool(name="ca2", bufs=2) as ca2, \
         tc.tile_pool(name="cp", bufs=4, space="PSUM") as cp:
        for b in range(BSH):
            comb = [caq.tile([128, T * J], BF16, tag=f"comb{q}", name=f"comb{b}_{q}") for q in range(4)]
            # ---------- temporal tail ----------
            gt = [caq.tile([128, TOKT_B], BF16, tag=f"gt{m}", name=f"gt{b}_{m}") for m in range(2)]
            for m in range(2):
                yl = ca2.tile([128, TOKT_B], BF16, tag="yl", name=f"yl{b}_{m}")
                nc.sync.dma_start(yl[:], yt_d[m][:, b * TOKT_B:(b + 1) * TOKT_B])
                nc.scalar.activation(gt[m][:], yl[:], AF.Gelu,
                                     scale=bnc[:, m:m + 1], bias=bnc[:, 2 + m:3 + m])
            zpad = [caq.tile([128, NT_B * 83], F32, tag=f"zpad{m}", name=f"zpad{b}_{m}") for m in range(2)]
            pchunks = [(0, 6), (6, 12), (12, 17)]
            for m in range(2):
                for (j0, j1) in pchunks:
                    ps = cp.tile([128, 512], F32, tag="cbig", name=f"cpw{b}_{m}_{j0}")
                    w = (j1 - j0) * LT
                    for k in range(2):
                        nc.tensor.matmul(
                            ps[:, :w],
                            wpwT[k][:, 128 * m:128 * (m + 1)],
                            gt[k][:, j0 * LT:j1 * LT],
                            start=(k == 0), stop=(k == 1))
                    dst = zpad[m][:].rearrange("p (j s) -> p j s", j=NT_B)[:, j0:j1, 1:82]
                    nc.scalar.copy(dst, ps[:, :w].rearrange("p (j t) -> p j t", j=j1 - j0))
                zv = zpad[m][:].rearrange("p (j s) -> p j s", j=NT_B)
                nc.vector.tensor_copy(zv[:, :, 0], zv[:, :, 1])
                nc.vector.tensor_copy(zv[:, :, 82], zv[:, :, 81])
                z23 = ca2.tile([128, TOKT_B], F32, tag="z23", name=f"z23{b}_{m}")
                nc.gpsimd.tensor_scalar_mul(z23[:].rearrange("p (j t) -> p j t", j=NT_B),
                                            zv[:, :, 1:82], 2.0 / 3.0)
                # out[t=3m+1] = z[m]; out[3m] = z[m-1]/3 + 2z[m]/3; out[3m+2] = z[m+1]/3 + 2z[m]/3
                z23v = z23[:].rearrange("p (j t) -> p j t", j=NT_B)
                dst1 = _interp_dst(comb[m], 1)
                nc.gpsimd.tensor_copy(dst1, zv[:, :, 1:82])
                dst0 = _interp_dst(comb[m], 0)
                nc.gpsimd.scalar_tensor_tensor(dst0, zv[:, :, 0:81], 1.0 / 3.0, z23v, OP.mult, OP.add)
                dst2 = _interp_dst(comb[m], 2)
                nc.gpsimd.scalar_tensor_tensor(dst2, zv[:, :, 2:83], 1.0 / 3.0, z23v, OP.mult, OP.add)
            # ---------- spatial tail ----------
            gs = [caq.tile([128, TOKS_B], BF16, tag=f"gs{m}", name=f"gs{b}_{m}") for m in range(2)]
            for m in range(2):
                yl = ca2.tile([128, TOKS_B], BF16, tag="ysl", name=f"ysl{b}_{m}")
                nc.sync.dma_start(yl[:], ys_d[m][:, b * TOKS_B:(b + 1) * TOKS_B])
                nc.scalar.activation(gs[m][:], yl[:], AF.Gelu,
                                     scale=bnc[:, 4 + m:5 + m], bias=bnc[:, 6 + m:7 + m])
            zs = [caq.tile([128, TOKS_B], F32, tag=f"zs{m}", name=f"zs{b}_{m}") for m in range(2)]
            nchunks = [(0, 61), (61, 122), (122, 183), (183, 243)]
            for m in range(2):
                for (n0, n1) in nchunks:
                    ps = cp.tile([128, 512], F32, tag="cbig", name=f"cps{b}_{m}_{n0}")
                    w = (n1 - n0) * LS
                    for k in range(2):
                        nc.tensor.matmul(
                            ps[:, :w],
                            wpwS[k][:, 128 * m:128 * (m + 1)],
                            gs[k][:, n0 * LS:n1 * LS],
                            start=(k == 0), stop=(k == 1))
                    nc.scalar.copy(zs[m][:, n0 * LS:n1 * LS], ps[:, :w])
                zsv = zs[m][:].rearrange("p (n l) -> p n l", n=NS_B)
                cmv = comb[2 + m][:].rearrange("p (t j) -> p t j", t=T)
                for jj in range(J):
                    lo, hi, w = int(lo_s[jj]), int(hi_s[jj]), float(w_s[jj])
                    if w < 1e-9 or lo == hi:
                        nc.gpsimd.tensor_copy(cmv[:, :, jj], zsv[:, :, lo])
                    else:
                        nc.gpsimd.tensor_scalar_mul(cmv[:, :, jj], zsv[:, :, lo], 1.0 - w)
                        nc.gpsimd.scalar_tensor_tensor(cmv[:, :, jj], zsv[:, :, hi], w,
                                                       cmv[:, :, jj], OP.mult, OP.add)
            if "comb" in dbg and b == 0:
                d = dbg_out("comb", [4, 128, T * J], BF16)
                for q in range(4):
                    nc.sync.dma_start(d[q], comb[q][:])

            # ---------- fusion MLP ----------
            g1T = [caq.tile([128, 4144], BF16, tag=f"g1T{k}", name=f"g1T{b}_{k}") for k in range(2)]
            NTOK = T * J  # 4131
            tchunks = [(i * 128, min(NTOK, (i + 1) * 128)) for i in range((NTOK + 127) // 128)]
            for (t0, t1) in tchunks:
                tl = t1 - t0
                ps = cp.tile([128, 256], F32, tag="ch1", name=f"h1{b}_{t0}")
                for q in range(4):
                    nc.tensor.matmul(ps[:tl, :], comb[q][:, t0:t1], fw1T[q][:],
                                     start=(q == 0), stop=(q == 3))
                bnst = ca2.tile([128, 6], F32, tag="bnst", name=f"bnst{b}_{t0}")
                nc.vector.bn_stats(bnst[:tl, :], ps[:tl, :])
                bnag = ca2.tile([128, 2], F32, tag="bnag", name=f"bnag{b}_{t0}")
                nc.vector.bn_aggr(bnag[:tl, :], bnst[:tl, :])
                r = ca2.tile([128, 1], F32, tag="lr", name=f"lr{b}_{t0}")
                sdv = ca2.tile([128, 1], F32, tag="sdv", name=f"sdv{b}_{t0}")
                nc.scalar.activation(sdv[:tl, :], bnag[:tl, 1:2], AF.Sqrt, bias=vecs[:tl, 24:25])
                nc.vector.reciprocal(r[:tl, :], sdv[:tl, :])
                nmr = ca2.tile([128, 1], F32, tag="nmr", name=f"nmr{b}_{t0}")
                nc.vector.tensor_scalar(out=nmr[:tl, :], in0=bnag[:tl, 0:1],
                                        scalar1=r[:tl, :], scalar2=-1.0,
                                        op0=OP.mult, op1=OP.mult)
                g1c = ca2.tile([128, 256], BF16, tag="g1c", name=f"g1c{b}_{t0}")
                nc.scalar.activation(g1c[:tl, :], ps[:tl, :], AF.Gelu,
                                     scale=r[:tl, :], bias=nmr[:tl, :])
                tpad = (tl + 15) // 16 * 16
                for k in range(2):
                    nc.sync.dma_start_transpose(g1T[k][:, t0:t0 + tpad],
                                                g1c[:tpad, 128 * k:128 * (k + 1)])
            h2T = [caq.tile([128, T * J], BF16, tag=f"h2T{m}", name=f"h2T{b}_{m}") for m in range(2)]
            fchunks = [(i * 512, min(NTOK, (i + 1) * 512)) for i in range((NTOK + 511) // 512)]
            for m in range(2):
                for (c0, c1) in fchunks:
                    ps = cp.tile([128, 512], F32, tag="cbig", name=f"h2{b}_{m}_{c0}")
                    for k in range(2):
                        nc.tensor.matmul(ps[:, :c1 - c0], fw2T[k][:, 128 * m:128 * (m + 1)],
                                         g1T[k][:, c0:c1], start=(k == 0), stop=(k == 1))
                    nc.vector.tensor_scalar_add(h2T[m][:, c0:c1], ps[:, :c1 - c0],
                                                vecs[:, 20 + m:21 + m])
            for m in range(2):
                for (c0, c1) in fchunks:
                    ps = cp.tile([128, 512], F32, tag="cbig", name=f"h3{b}_{m}_{c0}")
                    for k in range(2):
                        nc.tensor.matmul(ps[:, :c1 - c0], pwfT[k][:, 128 * m:128 * (m + 1)],
                                         h2T[k][:, c0:c1], start=(k == 0), stop=(k == 1))
                    ol = ca2.tile([128, 512], F32, tag="ol", name=f"ol{b}_{m}_{c0}")
                    nc.vector.tensor_scalar_add(ol[:, :c1 - c0], ps[:, :c1 - c0],
                                                vecs[:, 22 + m:23 + m])
                    dst = out_t[b].rearrange("t j c -> c (t j)")[128 * m:128 * (m + 1), c0:c1]
                    nc.sync.dma_start(dst, ol[:, :c1 - c0])

    dr_cm.__exit__(None, None, None)
    ex_cm.__exit__(None, None, None)


def _interp_dst(comb, delta):
    # comb [128, (t j)]: col(t=3mm+delta, j) = mm*51 + delta*17 + j
    v = comb[:].rearrange("p (mm s) -> p mm s", s=51)[:, :, delta * J:(delta + 1) * J]
    return v.transpose([0, 2, 1])  # -> [128, j:17, mm:81] to match z views


_dram_cache = {}


def _qsd(dr, b, m):
    key = ("qsd", m)
    if key not in _dram_cache:
        _dram_cache[key] = dr.tile([128, TOKS_B], BF16, name=f"qsd{m}")
    return _dram_cache[key]


def _od2(dr, k):
    key = ("od2", k)
    if key not in _dram_cache:
        _dram_cache[key] = dr.tile([128, TOKS_B], BF16, name=f"od2_{k}")
    return _dram_cache[key]


def prep_inputs(x, t_qkv, t_proj, t_dw, t_bn_g, t_bn_b, t_pw,
                s_qkv, s_proj, s_dw, s_bn_g, s_bn_b, s_pw,
                f_w1, f_b1, f_ln_g, f_ln_b, f_w2, f_b2, p_w, p_b):
    """Host-side: build per-core in_maps."""
    bf = ml_dtypes.bfloat16
    assert np.all(np.asarray(f_ln_g) == 1.0) and np.all(np.asarray(f_ln_b) == 0.0), \
        "general LayerNorm affine not emitted in this build"
    assert np.all(np.asarray(f_b1) == 0.0), "f_b1 != 0 not emitted in this build"
    common = {
        "wqkv_t": np.ascontiguousarray((np.asarray(t_qkv).T / 3.0).astype(bf)),
        "wqkv_s": np.ascontiguousarray((np.asarray(s_qkv).T / 2.0).astype(bf)),
        "wproj_t": np.ascontiguousarray(np.asarray(t_proj).T.astype(bf)),
        "wproj_s": np.ascontiguousarray(np.asarray(s_proj).T.astype(bf)),
        "wpw_t": np.ascontiguousarray(np.asarray(t_pw).T.astype(bf)),
        "wpw_s": np.ascontiguousarray(np.asarray(s_pw).T.astype(bf)),
        "fw1": np.ascontiguousarray(np.asarray(f_w1).T.astype(bf)),
        "fw2": np.ascontiguousarray(np.asarray(f_w2).T.astype(bf)),
        "pwf": np.ascontiguousarray(np.asarray(p_w).T.astype(bf)),
        "idn": np.eye(128, dtype=bf),
    }
    vecs = np.zeros((128, 26), np.float32)
    vecs[:, 24] = EPS
    tdw = np.asarray(t_dw).reshape(C, 3)
    sdw = np.asarray(s_dw).reshape(C, 3)
    for m in range(2):
        vecs[:, 3 * m:3 * m + 3] = tdw[128 * m:128 * (m + 1)]
        vecs[:, 6 + 3 * m:9 + 3 * m] = sdw[128 * m:128 * (m + 1)]
        vecs[:, 12 + m] = np.asarray(t_bn_g)[128 * m:128 * (m + 1)]
        vecs[:, 14 + m] = np.asarray(t_bn_b)[128 * m:128 * (m + 1)]
        vecs[:, 16 + m] = np.asarray(s_bn_g)[128 * m:128 * (m + 1)]
        vecs[:, 18 + m] = np.asarray(s_bn_b)[128 * m:128 * (m + 1)]
        vecs[:, 20 + m] = np.asarray(f_b2)[128 * m:128 * (m + 1)]
        vecs[:, 22 + m] = np.asarray(p_b)[128 * m:128 * (m + 1)]
    common["vecs"] = vecs
    x = np.asarray(x, dtype=np.float32)
    in_maps = []
    for i in range(N_CORES):
        m = dict(common)
        m["xs"] = np.ascontiguousarray(x[i * BSH:(i + 1) * BSH])
        in_maps.append(m)
    return in_maps


_BUILD_CACHE = {}


def _get_nc(dbg=()):
    key = tuple(sorted(dbg))
    if key not in _BUILD_CACHE:
        _dram_cache.clear()
        nc = bass.Bass(trn_type="TRN2", target_bir_lowering=False, num_devices=N_CORES)
        dbg_outs = build(nc, dbg)
        _BUILD_CACHE[key] = (nc, dbg_outs)
    return _BUILD_CACHE[key]


def run(inputs, dbg=(), trace=False):
    nc, dbg_outs = _get_nc(dbg)
    in_maps = prep_inputs(**inputs)
    res = run_bass_kernel_spmd(nc, in_maps, core_ids=list(range(N_CORES)), trace=trace)
    outs = [r["out"] for r in res.results]
    full = np.concatenate(outs, axis=0)
    return full, res, dbg_outs


def kernel(**inputs) -> np.ndarray:
    full, _, _ = run(inputs)
    return full.astype(np.float32)

